# revision 8
# baseline (speedup 1.0000x reference)
"""Trainium2 Bass kernel for an attention block (B=4, C=64, H=W=64).

reference:
    xf = x.reshape(B, C, N)                      # N = H*W = 4096
    qkv = w_qkv @ xf + b_qkv                     # [B, 3C, N]
    q, k, v = split(qkv)
    attn = softmax(q^T k / sqrt(C), axis=-1)     # [B, N, N]
    out = w_proj @ (v @ attn^T) + b_proj + x

Sharding: 8 cores = (batch sample, query half). Each core computes K and V
for its sample plus the attention output for its 2048 queries; no
collectives. The score matrix is produced transposed ([keys, queries]) so
the attn @ V contraction needs no transposes, and the softmax denominator
comes out of the same matmul via a ones-column appended to V^T. The
division by the denominator is applied after the output projection (it
commutes), using a contraction-dim-1 matmul to broadcast the reciprocal
row across partitions.
"""

import numpy as np

import concourse.bass as bass
import concourse.tile as tile
from concourse import mybir
from concourse.bass_utils import run_bass_kernel_spmd

B, C = 4, 64
N = 4096          # H*W tokens
QH = N // 2       # queries per core
QSZ = 512         # query block (one PSUM bank at f32)
NQB = QH // QSZ
MC = 128          # key chunk = scores partition dim
NMC = N // MC

_F32 = mybir.dt.float32
_F32R = mybir.dt.float32r
_EXP = mybir.ActivationFunctionType.Exp


def _r(ap):
    return ap.bitcast(_F32R)


def _split_excess_waits(nc):
    """walrus CoreV3 in this toolchain accepts at most 1 sync wait on
    CTRL-class instructions and 2 elsewhere; move extras onto NoOps."""

    for f in nc.m.functions:
        for bb in f.blocks:
            new_insts = []
            changed = False
            for inst in bb.instructions:
                si = inst.sync_info
                limit = 1
                if si is not None and si.on_wait and len(si.on_wait) > limit:
                    waits = list(si.on_wait)
                    extra, keep = waits[:-limit], waits[-limit:]
                    for w in extra:
                        nop = mybir.InstNoOp(name=nc.get_next_instruction_name())
                        nop.engine = inst.engine
                        nop.sync_info = mybir.SyncInfo(on_wait=[w], on_update=[])
                        nc.register_instruction(nop)
                        new_insts.append(nop)
                    si.on_wait = keep
                    changed = True
                new_insts.append(inst)
            if changed:
                bb.instructions = new_insts


def build_graph():
    nc = bass.Bass("TRN2", target_bir_lowering=False, debug=False)

    xkv_ext = nc.declare_dram_parameter("xkv", [C, N], _F32, isOutput=False)
    xq_ext = nc.declare_dram_parameter("xq", [C, QH], _F32, isOutput=False)
    wqkvT_ext = nc.declare_dram_parameter("w_qkvT", [C, 3 * C], _F32, isOutput=False)
    bqkv_ext = nc.declare_dram_parameter("b_qkv", [3 * C, 1], _F32, isOutput=False)
    wprojT_ext = nc.declare_dram_parameter("w_projT", [C, C], _F32, isOutput=False)
    bproj_ext = nc.declare_dram_parameter("b_proj", [C, 1], _F32, isOutput=False)
    ones_ext = nc.declare_dram_parameter("ones", [MC, C, 1], _F32, isOutput=False)
    beff_ext = nc.declare_dram_parameter("b_eff", [C, 1], _F32, isOutput=False)
    out_ext = nc.declare_dram_parameter("out", [C, QH], _F32, isOutput=True)

    with (
        nc.allow_low_precision(reason="float32r is 32-bit storage"),
        tile.TileContext(nc) as tc,
    ):
        with tc.tile_pool(name="consts", bufs=1) as consts:
            X = consts.tile([C, N], _F32R, tag="x")
            XQ = consts.tile([C, QH], _F32R, tag="xq")
            WQK = consts.tile([C, 2 * C], _F32R, tag="wqk")
            WV = consts.tile([C, C], _F32R, tag="wv")
            WP = consts.tile([C, C], _F32R, tag="wp")
            BQ = consts.tile([C, 1], _F32, tag="bq")
            BK = consts.tile([C, 1], _F32, tag="bk")
            BEFF = consts.tile([C, 1], _F32, tag="beff")
            ONES1 = consts.tile([1, C], _F32R, tag="ones1")
            Q = consts.tile([C, QH], _F32R, tag="q")
            K = consts.tile([C, N], _F32R, tag="k")
            VT = consts.tile([MC, NMC * 65], _F32R, tag="vt")

            nc.sync.dma_start(out=X, in_=_r(xkv_ext[:, :]))
            nc.sync.dma_start(out=XQ, in_=_r(xq_ext[:, :]))
            nc.sync.dma_start(out=WQK, in_=_r(wqkvT_ext[:, 0 : 2 * C]))
            nc.sync.dma_start(out=WV, in_=_r(wqkvT_ext[:, 2 * C : 3 * C]))
            nc.sync.dma_start(out=WP, in_=_r(wprojT_ext[:, :]))
            nc.sync.dma_start(out=BQ, in_=bqkv_ext[0:C, :])
            nc.sync.dma_start(out=BK, in_=bqkv_ext[C : 2 * C, :])
            nc.sync.dma_start(out=BEFF, in_=beff_ext[:, :])
            nc.sync.dma_start(out=ONES1, in_=_r(ones_ext[0:1, :, 0]))
            vt_ones = VT.rearrange("p (n c) -> p n c", c=65)[:, :, 64:65]
            nc.sync.dma_start(out=vt_ones, in_=_r(ones_ext[:, 0:NMC, :]))

            # ---- projections ----
            with tc.tile_pool(name="qkvp", bufs=2, space="PSUM") as qkvp:
                for j in range(N // 512):
                    ps = qkvp.tile([C, 512], _F32, tag="kp")
                    nc.tensor.matmul(
                        ps,
                        WQK[:, C : 2 * C],
                        X[:, j * 512 : (j + 1) * 512],
                        start=True,
                        stop=True,
                    )
                    nc.vector.tensor_scalar_add(
                        K[:, j * 512 : (j + 1) * 512], ps, BK
                    )
                for j in range(QH // 512):
                    ps = qkvp.tile([C, 512], _F32, tag="qp")
                    nc.tensor.matmul(
                        ps,
                        WQK[:, 0:C],
                        XQ[:, j * 512 : (j + 1) * 512],
                        start=True,
                        stop=True,
                    )
                    nc.vector.tensor_scalar_add(
                        Q[:, j * 512 : (j + 1) * 512], ps, BQ
                    )
                for m in range(NMC):
                    ps = qkvp.tile([MC, C], _F32, tag="vp")
                    nc.tensor.matmul(
                        ps,
                        X[:, m * MC : (m + 1) * MC],
                        WV,
                        start=True,
                        stop=True,
                    )
                    nc.vector.tensor_copy(VT[:, m * 65 : m * 65 + C], ps)

            # ---- attention ----
            with (
                tc.tile_pool(name="scp", bufs=2, space="PSUM") as scp,
                tc.tile_pool(name="avp", bufs=2, space="PSUM") as avp,
                tc.tile_pool(name="epp", bufs=2, space="PSUM") as epp,
                tc.tile_pool(name="ebuf", bufs=3) as ebuf,
                tc.tile_pool(name="obuf", bufs=2) as obuf,
            ):
                for qb in range(NQB):
                    q0 = qb * QSZ
                    pav = avp.tile([C + 1, QSZ], _F32, tag="av")
                    for m in range(NMC):
                        pss = scp.tile([MC, QSZ], _F32, tag="s")
                        nc.tensor.matmul(
                            pss,
                            K[:, m * MC : (m + 1) * MC],
                            Q[:, q0 : q0 + QSZ],
                            start=True,
                            stop=True,
                        )
                        E = ebuf.tile([MC, QSZ], _F32R, tag="e")
                        nc.scalar.activation(E, pss, _EXP, bias=0.0, scale=0.125)
                        nc.tensor.matmul(
                            pav,
                            VT[:, m * 65 : (m + 1) * 65],
                            E,
                            start=(m == 0),
                            stop=(m == NMC - 1),
                        )
                    U = obuf.tile([C + 1, QSZ], _F32, tag="u")
                    nc.vector.tensor_copy(U, pav)
                    R1 = obuf.tile([1, QSZ], _F32R, tag="r1")
                    nc.vector.reciprocal(R1, U[C : C + 1, :])
                    pb = epp.tile([C, QSZ], _F32, tag="b")
                    nc.tensor.matmul(pb, ONES1, R1, start=True, stop=True)
                    UN = obuf.tile([C, QSZ], _F32R, tag="un")
                    nc.vector.tensor_mul(UN, U[0:C, :], pb)
                    pp = epp.tile([C, QSZ], _F32, tag="p")
                    nc.tensor.matmul(pp, WP, UN, start=True, stop=True)
                    O = obuf.tile([C, QSZ], _F32, tag="o")
                    nc.vector.tensor_scalar_add(O, pp, BEFF)
                    nc.vector.tensor_add(O, O, XQ[:, q0 : q0 + QSZ])
                    nc.sync.dma_start(out=out_ext[:, q0 : q0 + QSZ], in_=O)

    _split_excess_waits(nc)
    return nc


_GRAPH_CACHE = {}


def _get_graph():
    if "nc" not in _GRAPH_CACHE:
        _GRAPH_CACHE["nc"] = build_graph()
    return _GRAPH_CACHE["nc"]


_ONES = np.ones((MC, C, 1), dtype=np.float32)


def make_in_maps(x, w_qkv, b_qkv, w_proj, b_proj):
    xf = np.ascontiguousarray(np.asarray(x, dtype=np.float32).reshape(B, C, N))
    w_qkvT = np.ascontiguousarray(np.asarray(w_qkv, dtype=np.float32).T)
    b_qkv2 = np.ascontiguousarray(np.asarray(b_qkv, dtype=np.float32).reshape(3 * C, 1))
    w_projT = np.ascontiguousarray(np.asarray(w_proj, dtype=np.float32).T)
    b_proj2 = np.ascontiguousarray(np.asarray(b_proj, dtype=np.float32).reshape(C, 1))
    # fold the v-bias through the output projection (constant weight prep):
    # out = w_proj @ (v + b_v) @ attn^T + b_proj = w_proj @ (v @ attn^T) + b_eff
    b_eff = (
        np.asarray(w_proj, dtype=np.float32) @ np.asarray(b_qkv, dtype=np.float32)[2 * C :]
        + np.asarray(b_proj, dtype=np.float32)
    ).reshape(C, 1).astype(np.float32)

    in_maps = []
    for core in range(8):
        b, h = divmod(core, 2)
        in_maps.append(
            {
                "xkv": xf[b],
                "xq": np.ascontiguousarray(xf[b][:, h * QH : (h + 1) * QH]),
                "w_qkvT": w_qkvT,
                "b_qkv": b_qkv2,
                "w_projT": w_projT,
                "b_proj": b_proj2,
                "ones": _ONES,
                "b_eff": b_eff,
            }
        )
    return in_maps


def kernel(x, w_qkv, b_qkv, w_proj, b_proj):
    x = np.asarray(x)
    nc = _get_graph()
    in_maps = make_in_maps(x, w_qkv, b_qkv, w_proj, b_proj)
    res = run_bass_kernel_spmd(nc, in_maps, core_ids=list(range(8)))
    out = np.empty((B, C, N), dtype=np.float32)
    for core in range(8):
        b, h = divmod(core, 2)
        out[b][:, h * QH : (h + 1) * QH] = res.results[core]["out"]
    return out.reshape(x.shape).astype(np.float32)


# revision 10
# speedup vs baseline: 1.0167x; 1.0167x over previous
"""Trainium2 Bass kernel for an attention block (B=4, C=64, H=W=64).

reference:
    xf = x.reshape(B, C, N)                      # N = H*W = 4096
    qkv = w_qkv @ xf + b_qkv                     # [B, 3C, N]
    q, k, v = split(qkv)
    attn = softmax(q^T k / sqrt(C), axis=-1)     # [B, N, N]
    out = w_proj @ (v @ attn^T) + b_proj + x

Sharding: 8 cores = (batch sample, query half). Each core computes K and V
for its sample plus the attention output for its 2048 queries; no
collectives. The score matrix is produced transposed ([keys, queries]) so
the attn @ V contraction needs no transposes, and the softmax denominator
comes out of the same matmul via a ones-column appended to V^T. The
output projection is folded into the V projection weights on the host
(w_vp = w_proj @ w_v), and the division by the softmax denominator is
applied after that projection (it commutes), using a contraction-dim-1
matmul to broadcast the reciprocal row across partitions. The v/proj
biases fold to a single per-channel vector b_eff = w_proj @ b_v + b_proj
because softmax rows sum to one.
"""

import numpy as np

import concourse.bass as bass
import concourse.tile as tile
from concourse import mybir
from concourse.bass_utils import run_bass_kernel_spmd

B, C = 4, 64
N = 4096          # H*W tokens
QH = N // 2       # queries per core
QSZ = 512         # query block (one PSUM bank at f32)
NQB = QH // QSZ
MC = 128          # key chunk = scores partition dim
NMC = N // MC

_F32 = mybir.dt.float32
_F32R = mybir.dt.float32r
_EXP = mybir.ActivationFunctionType.Exp


def _r(ap):
    return ap.bitcast(_F32R)


def _split_excess_waits(nc):
    """walrus CoreV3 in this toolchain accepts at most one sync wait per
    instruction; move extras onto NoOps spliced just before it."""
    for f in nc.m.functions:
        for bb in f.blocks:
            new_insts = []
            changed = False
            for inst in bb.instructions:
                si = inst.sync_info
                if si is not None and si.on_wait and len(si.on_wait) > 1:
                    waits = list(si.on_wait)
                    extra, keep = waits[:-1], waits[-1:]
                    for w in extra:
                        nop = mybir.InstNoOp(name=nc.get_next_instruction_name())
                        nop.engine = inst.engine
                        nop.sync_info = mybir.SyncInfo(on_wait=[w], on_update=[])
                        nc.register_instruction(nop)
                        new_insts.append(nop)
                    si.on_wait = keep
                    changed = True
                new_insts.append(inst)
            if changed:
                bb.instructions = new_insts


def build_graph():
    nc = bass.Bass("TRN2", target_bir_lowering=False, debug=False)

    xkv_ext = nc.declare_dram_parameter("xkv", [C, N], _F32, isOutput=False)
    xq_ext = nc.declare_dram_parameter("xq", [C, QH], _F32, isOutput=False)
    # w_qkT = w_qkv[0:2C].T ; w_vpT = (w_proj @ w_qkv[2C:3C]).T
    wqkT_ext = nc.declare_dram_parameter("w_qkT", [C, 2 * C], _F32, isOutput=False)
    wvpT_ext = nc.declare_dram_parameter("w_vpT", [C, C], _F32, isOutput=False)
    bqk_ext = nc.declare_dram_parameter("b_qk", [2 * C, 1], _F32, isOutput=False)
    beff_ext = nc.declare_dram_parameter("b_eff", [C, 1], _F32, isOutput=False)
    ones_ext = nc.declare_dram_parameter("ones", [MC, C, 1], _F32, isOutput=False)
    out_ext = nc.declare_dram_parameter("out", [C, QH], _F32, isOutput=True)

    with (
        nc.allow_low_precision(reason="float32r is 32-bit storage"),
        tile.TileContext(nc) as tc,
    ):
        with tc.tile_pool(name="consts", bufs=1) as consts:
            X = consts.tile([C, N], _F32R, tag="x")
            XQ = consts.tile([C, QH], _F32R, tag="xq")
            WQK = consts.tile([C, 2 * C], _F32R, tag="wqk")
            WVP = consts.tile([C, C], _F32R, tag="wvp")
            BQK = consts.tile([2 * C, 1], _F32, tag="bqk")
            BK = consts.tile([C, 1], _F32, tag="bk")
            BEFF = consts.tile([C, 1], _F32, tag="beff")
            ONES1 = consts.tile([1, C], _F32R, tag="ones1")
            Q = consts.tile([C, QH], _F32R, tag="q")
            K = consts.tile([C, N], _F32R, tag="k")
            VT = consts.tile([MC, NMC * 65], _F32R, tag="vt")

            # chunked input DMAs so projections can start early
            for j in range(8):
                nc.sync.dma_start(
                    out=X[:, j * 512 : (j + 1) * 512],
                    in_=_r(xkv_ext[:, j * 512 : (j + 1) * 512]),
                )
            for j in range(4):
                nc.sync.dma_start(
                    out=XQ[:, j * 512 : (j + 1) * 512],
                    in_=_r(xq_ext[:, j * 512 : (j + 1) * 512]),
                )
            nc.sync.dma_start(out=WQK, in_=_r(wqkT_ext[:, :]))
            nc.sync.dma_start(out=WVP, in_=_r(wvpT_ext[:, :]))
            nc.sync.dma_start(out=BQK, in_=bqk_ext[:, :])
            nc.sync.dma_start(out=BK, in_=bqk_ext[C : 2 * C, :])
            nc.sync.dma_start(out=BEFF, in_=beff_ext[:, :])
            nc.sync.dma_start(out=ONES1, in_=_r(ones_ext[0:1, :, 0]))
            vt_ones = VT.rearrange("p (n c) -> p n c", c=65)[:, :, 64:65]
            nc.sync.dma_start(out=vt_ones, in_=_r(ones_ext[:, 0:NMC, :]))

            # ---- projections ----
            with tc.tile_pool(name="qkvp", bufs=2, space="PSUM") as qkvp:
                # q over the local half
                for j in range(QH // 512):
                    lo, hi = j * 512, (j + 1) * 512
                    ps = qkvp.tile([C, 512], _F32, tag="qp")
                    nc.tensor.matmul(
                        ps, WQK[:, 0:C], XQ[:, lo:hi], start=True, stop=True
                    )
                    nc.vector.tensor_scalar_add(Q[:, lo:hi], ps, BQK[0:C, :])
                # k over all tokens
                for j in range(N // 512):
                    lo, hi = j * 512, (j + 1) * 512
                    ps = qkvp.tile([C, 512], _F32, tag="kp")
                    nc.tensor.matmul(
                        ps, WQK[:, C : 2 * C], X[:, lo:hi], start=True, stop=True
                    )
                    nc.vector.tensor_scalar_add(K[:, lo:hi], ps, BK)
                # projected v, transposed: VT[n, o] = sum_c x[c, n] w_vpT[c, o]
                for m in range(NMC):
                    ps = qkvp.tile([MC, C], _F32, tag="vp")
                    nc.tensor.matmul(
                        ps, X[:, m * MC : (m + 1) * MC], WVP, start=True, stop=True
                    )
                    nc.vector.tensor_copy(VT[:, m * 65 : m * 65 + C], ps)

            # ---- attention ----
            # NOTE: the local K half comes from XQ, the rest from X; the
            # score matmuls slice K contiguously over both halves.
            with (
                tc.tile_pool(name="scp", bufs=2, space="PSUM") as scp,
                tc.tile_pool(name="avp", bufs=2, space="PSUM") as avp,
                tc.tile_pool(name="epp", bufs=2, space="PSUM") as epp,
                tc.tile_pool(name="ebuf", bufs=3) as ebuf,
                tc.tile_pool(name="obuf", bufs=2) as obuf,
            ):
                for qb in range(NQB):
                    q0 = qb * QSZ
                    pav = avp.tile([C + 1, QSZ], _F32, tag="av")
                    for m in range(NMC):
                        pss = scp.tile([MC, QSZ], _F32, tag="s")
                        nc.tensor.matmul(
                            pss,
                            K[:, m * MC : (m + 1) * MC],
                            Q[:, q0 : q0 + QSZ],
                            start=True,
                            stop=True,
                        )
                        E = ebuf.tile([MC, QSZ], _F32R, tag="e")
                        nc.scalar.activation(E, pss, _EXP, bias=0.0, scale=0.125)
                        nc.tensor.matmul(
                            pav,
                            VT[:, m * 65 : (m + 1) * 65],
                            E,
                            start=(m == 0),
                            stop=(m == NMC - 1),
                        )
                    # epilogue: normalize (after the folded projection), add
                    # b_eff and the residual
                    U = obuf.tile([C + 1, QSZ], _F32, tag="u")
                    nc.vector.tensor_copy(U, pav)
                    R1 = obuf.tile([1, QSZ], _F32R, tag="r1")
                    nc.vector.reciprocal(R1, U[C : C + 1, :])
                    pb = epp.tile([C, QSZ], _F32, tag="b")
                    nc.tensor.matmul(pb, ONES1, R1, start=True, stop=True)
                    UN = obuf.tile([C, QSZ], _F32, tag="un")
                    nc.vector.tensor_mul(UN, U[0:C, :], pb)
                    O = obuf.tile([C, QSZ], _F32, tag="o")
                    nc.vector.scalar_tensor_tensor(
                        out=O,
                        in0=UN,
                        scalar=BEFF,
                        in1=XQ[:, q0 : q0 + QSZ],
                        op0=mybir.AluOpType.add,
                        op1=mybir.AluOpType.add,
                    )
                    nc.sync.dma_start(out=out_ext[:, q0 : q0 + QSZ], in_=O)

    _split_excess_waits(nc)
    return nc


_GRAPH_CACHE = {}


def _get_graph():
    if "nc" not in _GRAPH_CACHE:
        _GRAPH_CACHE["nc"] = build_graph()
    return _GRAPH_CACHE["nc"]


_ONES = np.ones((MC, C, 1), dtype=np.float32)


def make_in_maps(x, w_qkv, b_qkv, w_proj, b_proj):
    xf = np.ascontiguousarray(np.asarray(x, dtype=np.float32).reshape(B, C, N))
    w_qkv = np.asarray(w_qkv, dtype=np.float32)
    b_qkv = np.asarray(b_qkv, dtype=np.float32)
    w_proj = np.asarray(w_proj, dtype=np.float32)
    b_proj = np.asarray(b_proj, dtype=np.float32)

    w_qkT = np.ascontiguousarray(w_qkv[0 : 2 * C].T)
    # fold the output projection into the v projection (weight prep):
    # w_proj @ (w_v @ x) == (w_proj @ w_v) @ x
    w_vpT = np.ascontiguousarray((w_proj @ w_qkv[2 * C :]).T.astype(np.float32))
    b_qk = np.ascontiguousarray(b_qkv[0 : 2 * C].reshape(2 * C, 1))
    # v/proj biases fold to one vector because softmax rows sum to 1
    b_eff = (w_proj @ b_qkv[2 * C :] + b_proj).reshape(C, 1).astype(np.float32)

    in_maps = []
    for core in range(8):
        b, h = divmod(core, 2)
        in_maps.append(
            {
                "xkv": xf[b],
                "xq": np.ascontiguousarray(xf[b][:, h * QH : (h + 1) * QH]),
                "w_qkT": w_qkT,
                "w_vpT": w_vpT,
                "b_qk": b_qk,
                "b_eff": b_eff,
                "ones": _ONES,
            }
        )
    return in_maps


def kernel(x, w_qkv, b_qkv, w_proj, b_proj):
    x = np.asarray(x)
    nc = _get_graph()
    in_maps = make_in_maps(x, w_qkv, b_qkv, w_proj, b_proj)
    res = run_bass_kernel_spmd(nc, in_maps, core_ids=list(range(8)))
    out = np.empty((B, C, N), dtype=np.float32)
    for core in range(8):
        b, h = divmod(core, 2)
        out[b][:, h * QH : (h + 1) * QH] = res.results[core]["out"]
    return out.reshape(x.shape).astype(np.float32)


# revision 11
# speedup vs baseline: 1.1744x; 1.1551x over previous
"""Trainium2 Bass kernel for an attention block (B=4, C=64, H=W=64).

reference:
    xf = x.reshape(B, C, N)                      # N = H*W = 4096
    qkv = w_qkv @ xf + b_qkv                     # [B, 3C, N]
    q, k, v = split(qkv)
    attn = softmax(q^T k / sqrt(C), axis=-1)     # [B, N, N]
    out = w_proj @ (v @ attn^T) + b_proj + x

Sharding: 8 cores = (batch sample, query half). Each core computes K and V
for its sample plus the attention output for its 2048 queries; no
collectives. The score matrix is produced transposed ([keys, queries]) so
the attn @ V contraction needs no transposes, and the softmax denominator
comes out of the same matmul via a ones-column appended to V^T. The
output projection is folded into the V projection weights on the host
(w_vp = w_proj @ w_v), and the division by the softmax denominator is
applied after that projection (it commutes), using a contraction-dim-1
matmul to broadcast the reciprocal row across partitions. The v/proj
biases fold to a single per-channel vector b_eff = w_proj @ b_v + b_proj
because softmax rows sum to one.
"""

import numpy as np

import concourse.bass as bass
import concourse.tile as tile
from concourse import mybir
from concourse.bass_utils import run_bass_kernel_spmd

B, C = 4, 64
N = 4096          # H*W tokens
QH = N // 2       # queries per core
QSZ = 512         # query block (one PSUM bank at f32)
NQB = QH // QSZ
MC = 128          # key chunk = scores partition dim
NMC = N // MC

_F32 = mybir.dt.float32
_F32R = mybir.dt.float32r
_EXP = mybir.ActivationFunctionType.Exp


def _r(ap):
    return ap.bitcast(_F32R)


def _split_excess_waits(nc):
    """walrus CoreV3 in this toolchain accepts at most one sync wait per
    instruction; move extras onto NoOps spliced just before it."""
    for f in nc.m.functions:
        for bb in f.blocks:
            new_insts = []
            changed = False
            for inst in bb.instructions:
                si = inst.sync_info
                if si is not None and si.on_wait and len(si.on_wait) > 1:
                    waits = list(si.on_wait)
                    extra, keep = waits[:-1], waits[-1:]
                    for w in extra:
                        nop = mybir.InstNoOp(name=nc.get_next_instruction_name())
                        nop.engine = inst.engine
                        nop.sync_info = mybir.SyncInfo(on_wait=[w], on_update=[])
                        nc.register_instruction(nop)
                        new_insts.append(nop)
                    si.on_wait = keep
                    changed = True
                new_insts.append(inst)
            if changed:
                bb.instructions = new_insts


def build_graph():
    nc = bass.Bass("TRN2", target_bir_lowering=False, debug=False)

    xkv_ext = nc.declare_dram_parameter("xkv", [C, N], _F32, isOutput=False)
    xq_ext = nc.declare_dram_parameter("xq", [C, QH], _F32, isOutput=False)
    # w_qkT = w_qkv[0:2C].T ; w_vpT = (w_proj @ w_qkv[2C:3C]).T
    wqkT_ext = nc.declare_dram_parameter("w_qkT", [C, 2 * C], _F32, isOutput=False)
    wvpT_ext = nc.declare_dram_parameter("w_vpT", [C, C], _F32, isOutput=False)
    bqk_ext = nc.declare_dram_parameter("b_qk", [2 * C, 1], _F32, isOutput=False)
    beff_ext = nc.declare_dram_parameter("b_eff", [C, 1], _F32, isOutput=False)
    ones_ext = nc.declare_dram_parameter("ones", [MC, C, 1], _F32, isOutput=False)
    out_ext = nc.declare_dram_parameter("out", [C, QH], _F32, isOutput=True)

    QB = 1024      # scores/exp batch (2 PSUM banks)
    NQB2 = QH // QB
    with (
        nc.allow_low_precision(reason="float32r is 32-bit storage"),
        tile.TileContext(nc) as tc,
    ):
        with tc.tile_pool(name="consts", bufs=1) as consts:
            X = consts.tile([C, N], _F32R, tag="x")
            XQ = consts.tile([C, QH], _F32R, tag="xq")
            WQK = consts.tile([C, 2 * C], _F32R, tag="wqk")
            WVP = consts.tile([C, C], _F32R, tag="wvp")
            BQK = consts.tile([2 * C, 1], _F32, tag="bqk")
            BK = consts.tile([C, 1], _F32, tag="bk")
            BEFF = consts.tile([C, 1], _F32, tag="beff")
            ONES1 = consts.tile([1, C], _F32R, tag="ones1")
            Q = consts.tile([C, QH], _F32R, tag="q")
            K = consts.tile([C, N], _F32R, tag="k")
            VT = consts.tile([MC, NMC * 65], _F32R, tag="vt")

            # chunked input DMAs so projections can start early
            for j in range(8):
                nc.gpsimd.dma_start(
                    out=X[:, j * 512 : (j + 1) * 512],
                    in_=_r(xkv_ext[:, j * 512 : (j + 1) * 512]),
                )
            for j in range(4):
                nc.sync.dma_start(
                    out=XQ[:, j * 512 : (j + 1) * 512],
                    in_=_r(xq_ext[:, j * 512 : (j + 1) * 512]),
                )
            nc.sync.dma_start(out=WQK, in_=_r(wqkT_ext[:, :]))
            nc.sync.dma_start(out=WVP, in_=_r(wvpT_ext[:, :]))
            nc.sync.dma_start(out=BQK, in_=bqk_ext[:, :])
            nc.sync.dma_start(out=BK, in_=bqk_ext[C : 2 * C, :])
            nc.sync.dma_start(out=BEFF, in_=beff_ext[:, :])
            nc.sync.dma_start(out=ONES1, in_=_r(ones_ext[0:1, :, 0]))
            vt_ones = VT.rearrange("p (n c) -> p n c", c=65)[:, :, 64:65]
            nc.sync.dma_start(out=vt_ones, in_=_r(ones_ext[:, 0:NMC, :]))

            # ---- one shared PSUM pool: no pool-close barrier between the
            # projection stage and the attention loop, so they interleave ----
            with (
                tc.tile_pool(name="psum", bufs=2, space="PSUM") as psum,
                tc.tile_pool(name="ebuf", bufs=3) as ebuf,
                tc.tile_pool(name="obuf", bufs=2) as obuf,
            ):
                # q over the local half
                for j in range(QH // 512):
                    lo, hi = j * 512, (j + 1) * 512
                    ps = psum.tile([C, 512], _F32, tag="s")
                    nc.tensor.matmul(
                        ps, WQK[:, 0:C], XQ[:, lo:hi], start=True, stop=True
                    )
                    nc.vector.tensor_scalar_add(Q[:, lo:hi], ps, BQK[0:C, :])
                # k over all tokens
                for j in range(N // 512):
                    lo, hi = j * 512, (j + 1) * 512
                    ps = psum.tile([C, 512], _F32, tag="s")
                    nc.tensor.matmul(
                        ps, WQK[:, C : 2 * C], X[:, lo:hi], start=True, stop=True
                    )
                    nc.vector.tensor_scalar_add(K[:, lo:hi], ps, BK)
                # projected v, transposed: VT[n, o] = sum_c x[c, n] w_vpT[c, o]
                for m in range(NMC):
                    ps = psum.tile([MC, C], _F32, tag="av")
                    nc.tensor.matmul(
                        ps, X[:, m * MC : (m + 1) * MC], WVP, start=True, stop=True
                    )
                    nc.vector.tensor_copy(VT[:, m * 65 : m * 65 + C], ps)

                # ---- attention ----
                for qb in range(NQB2):
                    q0 = qb * QB
                    pav = psum.tile([C + 1, QB], _F32, tag="av")
                    for m in range(NMC):
                        pss = psum.tile([MC, QB], _F32, tag="s")
                        for h in (0, 512):
                            nc.tensor.matmul(
                                pss[:, h : h + 512],
                                K[:, m * MC : (m + 1) * MC],
                                Q[:, q0 + h : q0 + h + 512],
                                start=True,
                                stop=True,
                            )
                        E = ebuf.tile([MC, QB], _F32R, tag="e")
                        nc.scalar.activation(E, pss, _EXP, bias=0.0, scale=0.125)
                        for h in (0, 512):
                            nc.tensor.matmul(
                                pav[:, h : h + 512],
                                VT[:, m * 65 : (m + 1) * 65],
                                E[:, h : h + 512],
                                start=(m == 0),
                                stop=(m == NMC - 1),
                            )
                    # epilogue at 512 granularity: normalize (the output
                    # projection is folded into VT), add b_eff + residual
                    for h in (0, 512):
                        U = obuf.tile([C + 1, 512], _F32, tag="u")
                        nc.vector.tensor_copy(U, pav[:, h : h + 512])
                        R1 = obuf.tile([1, 512], _F32R, tag="r1")
                        nc.vector.reciprocal(R1, U[C : C + 1, :])
                        pb = psum.tile([C, 512], _F32, tag="s")
                        nc.tensor.matmul(pb, ONES1, R1, start=True, stop=True)
                        UN = obuf.tile([C, 512], _F32, tag="un")
                        nc.vector.tensor_mul(UN, U[0:C, :], pb)
                        O = obuf.tile([C, 512], _F32, tag="o")
                        nc.vector.scalar_tensor_tensor(
                            out=O,
                            in0=UN,
                            scalar=BEFF,
                            in1=XQ[:, q0 + h : q0 + h + 512],
                            op0=mybir.AluOpType.add,
                            op1=mybir.AluOpType.add,
                        )
                        nc.sync.dma_start(
                            out=out_ext[:, q0 + h : q0 + h + 512], in_=O
                        )

    _split_excess_waits(nc)
    return nc


_GRAPH_CACHE = {}


def _get_graph():
    if "nc" not in _GRAPH_CACHE:
        _GRAPH_CACHE["nc"] = build_graph()
    return _GRAPH_CACHE["nc"]


_ONES = np.ones((MC, C, 1), dtype=np.float32)


def make_in_maps(x, w_qkv, b_qkv, w_proj, b_proj):
    xf = np.ascontiguousarray(np.asarray(x, dtype=np.float32).reshape(B, C, N))
    w_qkv = np.asarray(w_qkv, dtype=np.float32)
    b_qkv = np.asarray(b_qkv, dtype=np.float32)
    w_proj = np.asarray(w_proj, dtype=np.float32)
    b_proj = np.asarray(b_proj, dtype=np.float32)

    w_qkT = np.ascontiguousarray(w_qkv[0 : 2 * C].T)
    # fold the output projection into the v projection (weight prep):
    # w_proj @ (w_v @ x) == (w_proj @ w_v) @ x
    w_vpT = np.ascontiguousarray((w_proj @ w_qkv[2 * C :]).T.astype(np.float32))
    b_qk = np.ascontiguousarray(b_qkv[0 : 2 * C].reshape(2 * C, 1))
    # v/proj biases fold to one vector because softmax rows sum to 1
    b_eff = (w_proj @ b_qkv[2 * C :] + b_proj).reshape(C, 1).astype(np.float32)

    in_maps = []
    for core in range(8):
        b, h = divmod(core, 2)
        in_maps.append(
            {
                "xkv": xf[b],
                "xq": np.ascontiguousarray(xf[b][:, h * QH : (h + 1) * QH]),
                "w_qkT": w_qkT,
                "w_vpT": w_vpT,
                "b_qk": b_qk,
                "b_eff": b_eff,
                "ones": _ONES,
            }
        )
    return in_maps


def kernel(x, w_qkv, b_qkv, w_proj, b_proj):
    x = np.asarray(x)
    nc = _get_graph()
    in_maps = make_in_maps(x, w_qkv, b_qkv, w_proj, b_proj)
    res = run_bass_kernel_spmd(nc, in_maps, core_ids=list(range(8)))
    out = np.empty((B, C, N), dtype=np.float32)
    for core in range(8):
        b, h = divmod(core, 2)
        out[b][:, h * QH : (h + 1) * QH] = res.results[core]["out"]
    return out.reshape(x.shape).astype(np.float32)


# revision 12
# speedup vs baseline: 1.3370x; 1.1384x over previous
"""Trainium2 Bass kernel for an attention block (B=4, C=64, H=W=64).

reference:
    xf = x.reshape(B, C, N)                      # N = H*W = 4096
    qkv = w_qkv @ xf + b_qkv                     # [B, 3C, N]
    q, k, v = split(qkv)
    attn = softmax(q^T k / sqrt(C), axis=-1)     # [B, N, N]
    out = w_proj @ (v @ attn^T) + b_proj + x

Sharding: 8 cores = (batch sample, query half). Each core receives its
sample's tokens ROTATED so its own 2048 queries are always columns
0:2048 (attention is permutation-invariant over keys, so K/V built from
the rotated layout give identical outputs). Each core computes K and V
for its sample plus the attention output for its queries; no
collectives.

The score matrix is produced transposed ([keys, queries]) so the
attn @ V contraction needs no transposes, and the softmax denominator
comes out of the same matmul via a ones-column appended to V^T. The
output projection is folded into the V projection weights on the host
(w_vp = w_proj @ w_v), and the division by the softmax denominator is
applied after that projection (it commutes), using a contraction-dim-1
matmul to broadcast the reciprocal row across partitions. The v/proj
biases fold to a single per-channel vector b_eff = w_proj @ b_v + b_proj
because softmax rows sum to one.
"""

import numpy as np

import concourse.bass as bass
import concourse.tile as tile
from concourse import mybir
from concourse.bass_utils import run_bass_kernel_spmd

B, C = 4, 64
N = 4096          # H*W tokens
QH = N // 2       # queries per core
QB = 1024         # scores/exp batch (2 PSUM banks)
NQB = QH // QB
MC = 128          # key chunk = scores partition dim
NMC = N // MC

_F32 = mybir.dt.float32
_F32R = mybir.dt.float32r
_EXP = mybir.ActivationFunctionType.Exp
_ADD = mybir.AluOpType.add


def _r(ap):
    return ap.bitcast(_F32R)


def _split_excess_waits(nc):
    """walrus CoreV3 in this toolchain accepts at most one sync wait per
    instruction; move extras onto NoOps spliced just before it."""
    for f in nc.m.functions:
        for bb in f.blocks:
            new_insts = []
            changed = False
            for inst in bb.instructions:
                si = inst.sync_info
                if si is not None and si.on_wait and len(si.on_wait) > 1:
                    waits = list(si.on_wait)
                    extra, keep = waits[:-1], waits[-1:]
                    for w in extra:
                        nop = mybir.InstNoOp(name=nc.get_next_instruction_name())
                        nop.engine = inst.engine
                        nop.sync_info = mybir.SyncInfo(on_wait=[w], on_update=[])
                        nc.register_instruction(nop)
                        new_insts.append(nop)
                    si.on_wait = keep
                    changed = True
                new_insts.append(inst)
            if changed:
                bb.instructions = new_insts


def build_graph():
    nc = bass.Bass("TRN2", target_bir_lowering=False, debug=False)

    x_ext = nc.declare_dram_parameter("x", [C, N], _F32, isOutput=False)
    # w_qkT = w_qkv[0:2C].T ; w_vpT = (w_proj @ w_qkv[2C:3C]).T
    wqkT_ext = nc.declare_dram_parameter("w_qkT", [C, 2 * C], _F32, isOutput=False)
    wvpT_ext = nc.declare_dram_parameter("w_vpT", [C, C], _F32, isOutput=False)
    bqk_ext = nc.declare_dram_parameter("b_qk", [2 * C, 1], _F32, isOutput=False)
    beff_ext = nc.declare_dram_parameter("b_eff", [C, 1], _F32, isOutput=False)
    ones_ext = nc.declare_dram_parameter("ones", [MC, C, 1], _F32, isOutput=False)
    out_ext = nc.declare_dram_parameter("out", [C, QH], _F32, isOutput=True)

    with (
        nc.allow_low_precision(reason="float32r is 32-bit storage"),
        tile.TileContext(nc) as tc,
        tc.tile_pool(name="consts", bufs=1) as consts,
        # PSUM budget (8 banks): s 2x[128,1024]=4, av 1x[65,1024]=2,
        # pj 2x[<=128,512]=2. qkv psums and the epilogue broadcast share pj.
        tc.tile_pool(name="spool", bufs=2, space="PSUM") as spool,
        tc.tile_pool(name="avpool", bufs=1, space="PSUM") as avpool,
        tc.tile_pool(name="pjpool", bufs=2, space="PSUM") as pjpool,
        tc.tile_pool(name="ebuf", bufs=3) as ebuf,
        tc.tile_pool(name="obuf", bufs=2) as obuf,
    ):
        X = consts.tile([C, N], _F32R, tag="x")
        WQK = consts.tile([C, 2 * C], _F32R, tag="wqk")
        WVP = consts.tile([C, C], _F32R, tag="wvp")
        BQK = consts.tile([2 * C, 1], _F32, tag="bqk")
        BK = consts.tile([C, 1], _F32, tag="bk")
        BEFF = consts.tile([C, 1], _F32, tag="beff")
        ONES1 = consts.tile([1, C], _F32R, tag="ones1")
        Q = consts.tile([C, QH], _F32R, tag="q")
        K = consts.tile([C, N], _F32R, tag="k")
        VT = consts.tile([MC, NMC * 65], _F32R, tag="vt")
        VT3 = VT.rearrange("p (n c) -> p n c", c=65)

        # weights/biases first, then x chunks across two DMA queues
        nc.sync.dma_start(out=WQK, in_=_r(wqkT_ext[:, :]))
        nc.sync.dma_start(out=WVP, in_=_r(wvpT_ext[:, :]))
        nc.sync.dma_start(out=BQK, in_=bqk_ext[:, :])
        nc.sync.dma_start(out=BK, in_=bqk_ext[C : 2 * C, :])
        nc.sync.dma_start(out=BEFF, in_=beff_ext[:, :])
        nc.sync.dma_start(out=ONES1, in_=_r(ones_ext[0:1, :, 0]))
        nc.sync.dma_start(out=VT3[:, :, 64:65], in_=_r(ones_ext[:, 0:NMC, :]))
        for j in range(8):
            eng = nc.gpsimd if j % 2 == 0 else nc.scalar
            eng.dma_start(
                out=X[:, j * 512 : (j + 1) * 512],
                in_=_r(x_ext[:, j * 512 : (j + 1) * 512]),
            )

        # ---- projections (interleave with the attention loop; separate
        # psum rings keep the score matmuls off the projection ring) ----
        for j in range(QH // 512):
            lo, hi = j * 512, (j + 1) * 512
            ps = pjpool.tile([C, 512], _F32, tag="pj")
            nc.tensor.matmul(ps, WQK[:, 0:C], X[:, lo:hi], start=True, stop=True)
            nc.vector.tensor_scalar_add(Q[:, lo:hi], ps, BQK[0:C, :])
        for j in range(N // 512):
            lo, hi = j * 512, (j + 1) * 512
            ps = pjpool.tile([C, 512], _F32, tag="pj")
            nc.tensor.matmul(
                ps, WQK[:, C : 2 * C], X[:, lo:hi], start=True, stop=True
            )
            nc.vector.tensor_scalar_add(K[:, lo:hi], ps, BK)
        # projected v, transposed, 4 chunks per psum tile + 1 strided copy
        for g in range(NMC // 4):
            ps = pjpool.tile([MC, 4, C], _F32, tag="pj")
            for i in range(4):
                m = g * 4 + i
                nc.tensor.matmul(
                    ps[:, i, :],
                    X[:, m * MC : (m + 1) * MC],
                    WVP,
                    start=True,
                    stop=True,
                )
            nc.vector.tensor_copy(VT3[:, g * 4 : (g + 1) * 4, 0:C], ps)

        # ---- attention ----
        def epilogue(qb, pav, esz, ucopy_engine):
            q0 = qb * QB
            for h in range(0, QB, esz):
                U = obuf.tile([C + 1, esz], _F32, tag="u")
                ucopy_engine(U, pav[:, h : h + esz])
                R1 = obuf.tile([1, esz], _F32R, tag="r1")
                nc.vector.reciprocal(R1, U[C : C + 1, :])
                pb = pjpool.tile([C, esz], _F32, tag="pj")
                nc.tensor.matmul(pb, ONES1, R1, start=True, stop=True)
                UN = obuf.tile([C, esz], _F32, tag="un")
                nc.vector.tensor_mul(UN, U[0:C, :], pb)
                O = obuf.tile([C, esz], _F32, tag="o")
                nc.vector.scalar_tensor_tensor(
                    out=O,
                    in0=UN,
                    scalar=BEFF,
                    in1=X[:, q0 + h : q0 + h + esz],
                    op0=_ADD,
                    op1=_ADD,
                )
                nc.sync.dma_start(out=out_ext[:, q0 + h : q0 + h + esz], in_=O)

        pending = None  # (qb, pav) whose epilogue is emitted into the next block
        for qb in range(NQB):
            q0 = qb * QB
            pav = avpool.tile([C + 1, QB], _F32, tag="av")
            for m in range(NMC):
                pss = spool.tile([MC, QB], _F32, tag="s")
                for h in (0, 512):
                    nc.tensor.matmul(
                        pss[:, h : h + 512],
                        K[:, m * MC : (m + 1) * MC],
                        Q[:, q0 + h : q0 + h + 512],
                        start=True,
                        stop=True,
                    )
                E = ebuf.tile([MC, QB], _F32R, tag="e")
                nc.scalar.activation(E, pss, _EXP, bias=0.0, scale=0.125)
                for h in (0, 512):
                    nc.tensor.matmul(
                        pav[:, h : h + 512],
                        VT[:, m * 65 : (m + 1) * 65],
                        E[:, h : h + 512],
                        start=(m == 0),
                        stop=(m == NMC - 1),
                    )
                if m == 4 and pending is not None:
                    # previous block's epilogue rides behind this block's
                    # first few iterations in every engine queue
                    epilogue(*pending, esz=512, ucopy_engine=nc.vector.tensor_copy)
                    pending = None
            pending = (qb, pav)
        # final epilogue: fine-grained and with the psum->sbuf copy on the
        # (now idle) scalar engine so the DVE chain pipelines
        epilogue(*pending, esz=256, ucopy_engine=nc.scalar.copy)

    _split_excess_waits(nc)
    return nc


_GRAPH_CACHE = {}


def _get_graph():
    if "nc" not in _GRAPH_CACHE:
        _GRAPH_CACHE["nc"] = build_graph()
    return _GRAPH_CACHE["nc"]


_ONES = np.ones((MC, C, 1), dtype=np.float32)


def make_in_maps(x, w_qkv, b_qkv, w_proj, b_proj):
    xf = np.ascontiguousarray(np.asarray(x, dtype=np.float32).reshape(B, C, N))
    w_qkv = np.asarray(w_qkv, dtype=np.float32)
    b_qkv = np.asarray(b_qkv, dtype=np.float32)
    w_proj = np.asarray(w_proj, dtype=np.float32)
    b_proj = np.asarray(b_proj, dtype=np.float32)

    w_qkT = np.ascontiguousarray(w_qkv[0 : 2 * C].T)
    # fold the output projection into the v projection (weight prep):
    # w_proj @ (w_v @ x) == (w_proj @ w_v) @ x
    w_vpT = np.ascontiguousarray((w_proj @ w_qkv[2 * C :]).T.astype(np.float32))
    b_qk = np.ascontiguousarray(b_qkv[0 : 2 * C].reshape(2 * C, 1))
    # v/proj biases fold to one vector because softmax rows sum to 1
    b_eff = (w_proj @ b_qkv[2 * C :] + b_proj).reshape(C, 1).astype(np.float32)

    in_maps = []
    for core in range(8):
        b, h = divmod(core, 2)
        # rotate tokens so this core's queries are columns 0:QH
        xr = np.ascontiguousarray(np.roll(xf[b], -h * QH, axis=1))
        in_maps.append(
            {
                "x": xr,
                "w_qkT": w_qkT,
                "w_vpT": w_vpT,
                "b_qk": b_qk,
                "b_eff": b_eff,
                "ones": _ONES,
            }
        )
    return in_maps


def kernel(x, w_qkv, b_qkv, w_proj, b_proj):
    x = np.asarray(x)
    nc = _get_graph()
    in_maps = make_in_maps(x, w_qkv, b_qkv, w_proj, b_proj)
    res = run_bass_kernel_spmd(nc, in_maps, core_ids=list(range(8)))
    out = np.empty((B, C, N), dtype=np.float32)
    for core in range(8):
        b, h = divmod(core, 2)
        out[b][:, h * QH : (h + 1) * QH] = res.results[core]["out"]
    return out.reshape(x.shape).astype(np.float32)


# revision 14
# speedup vs baseline: 1.4009x; 1.0479x over previous
"""Trainium2 Bass kernel for an attention block (B=4, C=64, H=W=64).

reference:
    xf = x.reshape(B, C, N)                      # N = H*W = 4096
    qkv = w_qkv @ xf + b_qkv                     # [B, 3C, N]
    q, k, v = split(qkv)
    attn = softmax(q^T k / sqrt(C), axis=-1)     # [B, N, N]
    out = w_proj @ (v @ attn^T) + b_proj + x

Sharding: 8 cores = (batch sample, query half). Each core receives its
sample's tokens ROTATED so its own 2048 queries are always columns
0:2048 (attention is permutation-invariant over keys, so K/V built from
the rotated layout give identical outputs). Each core computes K and V
for its sample plus the attention output for its queries; no
collectives.

The score matrix is produced transposed ([keys, queries]) so the
attn @ V contraction needs no transposes, and the softmax denominator
comes out of the same matmul via a ones-column appended to V^T. The
output projection is folded into the V projection weights on the host
(w_vp = w_proj @ w_v), and the division by the softmax denominator is
applied after that projection (it commutes), using a contraction-dim-1
matmul to broadcast the reciprocal row across partitions. The v/proj
biases fold to a single per-channel vector b_eff = w_proj @ b_v + b_proj
because softmax rows sum to one.
"""

import numpy as np

import concourse.bass as bass
import concourse.tile as tile
from concourse import mybir
from concourse.bass_utils import run_bass_kernel_spmd

B, C = 4, 64
N = 4096          # H*W tokens
QH = N // 2       # queries per core
QB = 1024         # scores/exp batch (2 PSUM banks)
NQB = QH // QB
MC = 128          # key chunk = scores partition dim
NMC = N // MC

_F32 = mybir.dt.float32
_F32R = mybir.dt.float32r
_EXP = mybir.ActivationFunctionType.Exp
_ADD = mybir.AluOpType.add


def _r(ap):
    return ap.bitcast(_F32R)


def _split_excess_waits(nc):
    """walrus CoreV3 in this toolchain accepts at most one sync wait per
    instruction; move extras onto NoOps spliced just before it."""
    for f in nc.m.functions:
        for bb in f.blocks:
            new_insts = []
            changed = False
            for inst in bb.instructions:
                si = inst.sync_info
                if si is not None and si.on_wait and len(si.on_wait) > 1:
                    waits = list(si.on_wait)
                    extra, keep = waits[:-1], waits[-1:]
                    for w in extra:
                        nop = mybir.InstNoOp(name=nc.get_next_instruction_name())
                        nop.engine = inst.engine
                        nop.sync_info = mybir.SyncInfo(on_wait=[w], on_update=[])
                        nc.register_instruction(nop)
                        new_insts.append(nop)
                    si.on_wait = keep
                    changed = True
                new_insts.append(inst)
            if changed:
                bb.instructions = new_insts


def build_graph():
    nc = bass.Bass("TRN2", target_bir_lowering=False, debug=False)

    x_ext = nc.declare_dram_parameter("x", [C, N], _F32, isOutput=False)
    # w_qkT = w_qkv[0:2C].T ; w_vpT = (w_proj @ w_qkv[2C:3C]).T
    wqkT_ext = nc.declare_dram_parameter("w_qkT", [C, 2 * C], _F32, isOutput=False)
    wvpT_ext = nc.declare_dram_parameter("w_vpT", [C, C], _F32, isOutput=False)
    bqk_ext = nc.declare_dram_parameter("b_qk", [2 * C, 1], _F32, isOutput=False)
    beff_ext = nc.declare_dram_parameter("b_eff", [C, 1], _F32, isOutput=False)
    ones_ext = nc.declare_dram_parameter("ones", [MC, C, 1], _F32, isOutput=False)
    out_ext = nc.declare_dram_parameter("out", [C, QH], _F32, isOutput=True)

    with (
        nc.allow_low_precision(reason="float32r is 32-bit storage"),
        tile.TileContext(nc) as tc,
        tc.tile_pool(name="consts", bufs=1) as consts,
        # PSUM budget (8 banks): s 2x[128,1024]=4, av 1x[65,1024]=2,
        # pj 2x[<=128,512]=2. qkv psums and the epilogue broadcast share pj.
        tc.tile_pool(name="spool", bufs=2, space="PSUM") as spool,
        tc.tile_pool(name="avpool", bufs=1, space="PSUM") as avpool,
        tc.tile_pool(name="pjpool", bufs=2, space="PSUM") as pjpool,
        tc.tile_pool(name="ebuf", bufs=3) as ebuf,
        tc.tile_pool(name="obuf", bufs=2) as obuf,
    ):
        X = consts.tile([C, N], _F32R, tag="x")
        WQK = consts.tile([C, 2 * C], _F32R, tag="wqk")
        WVP = consts.tile([C, C], _F32R, tag="wvp")
        BQK = consts.tile([2 * C, 1], _F32, tag="bqk")
        BK = consts.tile([C, 1], _F32, tag="bk")
        BEFF = consts.tile([C, 1], _F32, tag="beff")
        ONES1 = consts.tile([1, C], _F32R, tag="ones1")
        Q = consts.tile([C, QH], _F32R, tag="q")
        K = consts.tile([C, N], _F32R, tag="k")
        VT = consts.tile([MC, NMC * 65], _F32R, tag="vt")
        VT3 = VT.rearrange("p (n c) -> p n c", c=65)

        # weights/biases lead each DMA queue, then x chunks interleave
        # across the gpsimd and sync queues
        nc.gpsimd.dma_start(out=WVP, in_=_r(wvpT_ext[:, :]))
        nc.sync.dma_start(out=WQK, in_=_r(wqkT_ext[:, :]))
        nc.sync.dma_start(out=BQK, in_=bqk_ext[:, :])
        nc.sync.dma_start(out=BK, in_=bqk_ext[C : 2 * C, :])
        nc.sync.dma_start(out=BEFF, in_=beff_ext[:, :])
        nc.sync.dma_start(out=ONES1, in_=_r(ones_ext[0:1, :, 0]))
        nc.sync.dma_start(out=VT3[:, :, 64:65], in_=_r(ones_ext[:, 0:NMC, :]))
        for j in range(8):
            eng = nc.gpsimd if j % 2 == 0 else nc.sync
            eng.dma_start(
                out=X[:, j * 512 : (j + 1) * 512],
                in_=_r(x_ext[:, j * 512 : (j + 1) * 512]),
            )
        # preload the Exp table (1283ns) while DMAs are in flight
        WARM = consts.tile([1, 1], _F32, tag="warm")
        nc.vector.memset(WARM, 0.0)
        nc.scalar.activation(WARM, WARM, _EXP, bias=0.0, scale=1.0)

        # ---- projections, emitted just-in-time inside the attention loop
        # so no engine queue stalls on a not-yet-DMAed x chunk ----
        def emit_qp(j):
            lo, hi = j * 512, (j + 1) * 512
            ps = pjpool.tile([C, 512], _F32, tag="pj")
            nc.tensor.matmul(ps, WQK[:, 0:C], X[:, lo:hi], start=True, stop=True)
            nc.vector.tensor_scalar_add(Q[:, lo:hi], ps, BQK[0:C, :])

        def emit_kp(j):
            lo, hi = j * 512, (j + 1) * 512
            ps = pjpool.tile([C, 512], _F32, tag="pj")
            nc.tensor.matmul(
                ps, WQK[:, C : 2 * C], X[:, lo:hi], start=True, stop=True
            )
            nc.vector.tensor_scalar_add(K[:, lo:hi], ps, BK)

        def emit_vp(g):
            # projected v, transposed, 4 chunks per psum tile + 1 strided copy
            ps = pjpool.tile([MC, 4, C], _F32, tag="pj")
            for i in range(4):
                m = g * 4 + i
                nc.tensor.matmul(
                    ps[:, i, :],
                    X[:, m * MC : (m + 1) * MC],
                    WVP,
                    start=True,
                    stop=True,
                )
            nc.vector.tensor_copy(VT3[:, g * 4 : (g + 1) * 4, 0:C], ps)

        emit_qp(0)
        emit_qp(1)
        emit_kp(0)
        emit_vp(0)
        # one iteration of headroom: projections for chunk group j+1 are
        # emitted after main-loop iteration 4*j
        hooks = {}
        for j in range(1, 8):
            hooks.setdefault(4 * (j - 1), []).append((emit_kp, j))
        for g in range(1, 8):
            hooks.setdefault(4 * (g - 1) + 1, []).append((emit_vp, g))
        hooks.setdefault(2, []).append((emit_qp, 2))
        hooks.setdefault(3, []).append((emit_qp, 3))

        # ---- attention ----
        def epilogue(qb, pav, esz, ucopy_engine):
            q0 = qb * QB
            for h in range(0, QB, esz):
                U = obuf.tile([C + 1, esz], _F32, tag="u")
                ucopy_engine(U, pav[:, h : h + esz])
                R1 = obuf.tile([1, esz], _F32R, tag="r1")
                nc.vector.reciprocal(R1, U[C : C + 1, :])
                pb = pjpool.tile([C, esz], _F32, tag="pj")
                nc.tensor.matmul(pb, ONES1, R1, start=True, stop=True)
                UN = obuf.tile([C, esz], _F32, tag="un")
                nc.vector.tensor_mul(UN, U[0:C, :], pb)
                O = obuf.tile([C, esz], _F32, tag="o")
                nc.vector.scalar_tensor_tensor(
                    out=O,
                    in0=UN,
                    scalar=BEFF,
                    in1=X[:, q0 + h : q0 + h + esz],
                    op0=_ADD,
                    op1=_ADD,
                )
                nc.sync.dma_start(out=out_ext[:, q0 + h : q0 + h + esz], in_=O)

        pending = None  # (qb, pav) whose epilogue is emitted into the next block
        for qb in range(NQB):
            q0 = qb * QB
            pav = avpool.tile([C + 1, QB], _F32, tag="av")
            for m in range(NMC):
                pss = spool.tile([MC, QB], _F32, tag="s")
                for h in (0, 512):
                    nc.tensor.matmul(
                        pss[:, h : h + 512],
                        K[:, m * MC : (m + 1) * MC],
                        Q[:, q0 + h : q0 + h + 512],
                        start=True,
                        stop=True,
                    )
                E = ebuf.tile([MC, QB], _F32R, tag="e")
                nc.scalar.activation(E, pss, _EXP, bias=0.0, scale=0.125)
                for h in (0, 512):
                    nc.tensor.matmul(
                        pav[:, h : h + 512],
                        VT[:, m * 65 : (m + 1) * 65],
                        E[:, h : h + 512],
                        start=(m == 0),
                        stop=(m == NMC - 1),
                    )
                if qb == 0:
                    for fn, arg in hooks.get(m, ()):
                        fn(arg)
                if m == 4 and pending is not None:
                    # previous block's epilogue rides behind this block's
                    # first few iterations in every engine queue
                    epilogue(*pending, esz=512, ucopy_engine=nc.vector.tensor_copy)
                    pending = None
            pending = (qb, pav)
        # final epilogue: fine-grained and with the psum->sbuf copy on the
        # (now idle) scalar engine so the DVE chain pipelines
        epilogue(*pending, esz=256, ucopy_engine=nc.scalar.copy)

    _split_excess_waits(nc)
    return nc


_GRAPH_CACHE = {}


def _get_graph():
    if "nc" not in _GRAPH_CACHE:
        _GRAPH_CACHE["nc"] = build_graph()
    return _GRAPH_CACHE["nc"]


_ONES = np.ones((MC, C, 1), dtype=np.float32)


def make_in_maps(x, w_qkv, b_qkv, w_proj, b_proj):
    xf = np.ascontiguousarray(np.asarray(x, dtype=np.float32).reshape(B, C, N))
    w_qkv = np.asarray(w_qkv, dtype=np.float32)
    b_qkv = np.asarray(b_qkv, dtype=np.float32)
    w_proj = np.asarray(w_proj, dtype=np.float32)
    b_proj = np.asarray(b_proj, dtype=np.float32)

    w_qkT = np.ascontiguousarray(w_qkv[0 : 2 * C].T)
    # fold the output projection into the v projection (weight prep):
    # w_proj @ (w_v @ x) == (w_proj @ w_v) @ x
    w_vpT = np.ascontiguousarray((w_proj @ w_qkv[2 * C :]).T.astype(np.float32))
    b_qk = np.ascontiguousarray(b_qkv[0 : 2 * C].reshape(2 * C, 1))
    # v/proj biases fold to one vector because softmax rows sum to 1
    b_eff = (w_proj @ b_qkv[2 * C :] + b_proj).reshape(C, 1).astype(np.float32)

    in_maps = []
    for core in range(8):
        b, h = divmod(core, 2)
        # rotate tokens so this core's queries are columns 0:QH
        xr = np.ascontiguousarray(np.roll(xf[b], -h * QH, axis=1))
        in_maps.append(
            {
                "x": xr,
                "w_qkT": w_qkT,
                "w_vpT": w_vpT,
                "b_qk": b_qk,
                "b_eff": b_eff,
                "ones": _ONES,
            }
        )
    return in_maps


def kernel(x, w_qkv, b_qkv, w_proj, b_proj):
    x = np.asarray(x)
    nc = _get_graph()
    in_maps = make_in_maps(x, w_qkv, b_qkv, w_proj, b_proj)
    res = run_bass_kernel_spmd(nc, in_maps, core_ids=list(range(8)))
    out = np.empty((B, C, N), dtype=np.float32)
    for core in range(8):
        b, h = divmod(core, 2)
        out[b][:, h * QH : (h + 1) * QH] = res.results[core]["out"]
    return out.reshape(x.shape).astype(np.float32)


# revision 15
# speedup vs baseline: 1.4314x; 1.0217x over previous
"""Trainium2 Bass kernel for an attention block (B=4, C=64, H=W=64).

reference:
    xf = x.reshape(B, C, N)                      # N = H*W = 4096
    qkv = w_qkv @ xf + b_qkv                     # [B, 3C, N]
    q, k, v = split(qkv)
    attn = softmax(q^T k / sqrt(C), axis=-1)     # [B, N, N]
    out = w_proj @ (v @ attn^T) + b_proj + x

Sharding: 8 cores = (batch sample, query half). Each core receives its
sample's tokens ROTATED so its own 2048 queries are always columns
0:2048 (attention is permutation-invariant over keys, so K/V built from
the rotated layout give identical outputs). Each core computes K and V
for its sample plus the attention output for its queries; no
collectives.

The score matrix is produced transposed ([keys, queries]) so the
attn @ V contraction needs no transposes, and the softmax denominator
comes out of the same matmul via a ones-column appended to V^T. The
output projection is folded into the V projection weights on the host
(w_vp = w_proj @ w_v), and the division by the softmax denominator is
applied after that projection (it commutes), using a contraction-dim-1
matmul to broadcast the reciprocal row across partitions. The v/proj
biases fold to a single per-channel vector b_eff = w_proj @ b_v + b_proj
because softmax rows sum to one.
"""

import numpy as np

import concourse.bass as bass
import concourse.tile as tile
from concourse import mybir
from concourse.bass_utils import run_bass_kernel_spmd

B, C = 4, 64
N = 4096          # H*W tokens
QH = N // 2       # queries per core
QB = 1024         # scores/exp batch (2 PSUM banks)
NQB = QH // QB
MC = 128          # key chunk = scores partition dim
NMC = N // MC

_F32 = mybir.dt.float32
_F32R = mybir.dt.float32r
_EXP = mybir.ActivationFunctionType.Exp
_ADD = mybir.AluOpType.add


def _r(ap):
    return ap.bitcast(_F32R)


def _split_excess_waits(nc):
    """walrus CoreV3 in this toolchain accepts at most one sync wait per
    instruction; move extras onto NoOps spliced just before it."""
    for f in nc.m.functions:
        for bb in f.blocks:
            new_insts = []
            changed = False
            for inst in bb.instructions:
                si = inst.sync_info
                if si is not None and si.on_wait and len(si.on_wait) > 1:
                    waits = list(si.on_wait)
                    extra, keep = waits[:-1], waits[-1:]
                    for w in extra:
                        nop = mybir.InstNoOp(name=nc.get_next_instruction_name())
                        nop.engine = inst.engine
                        nop.sync_info = mybir.SyncInfo(on_wait=[w], on_update=[])
                        nc.register_instruction(nop)
                        new_insts.append(nop)
                    si.on_wait = keep
                    changed = True
                new_insts.append(inst)
            if changed:
                bb.instructions = new_insts


def build_graph():
    nc = bass.Bass("TRN2", target_bir_lowering=False, debug=False)

    x_ext = nc.declare_dram_parameter("x", [C, N], _F32, isOutput=False)
    # w_qkT = w_qkv[0:2C].T ; w_vpT = (w_proj @ w_qkv[2C:3C]).T
    wqkT_ext = nc.declare_dram_parameter("w_qkT", [C, 2 * C], _F32, isOutput=False)
    wvpT_ext = nc.declare_dram_parameter("w_vpT", [C, C], _F32, isOutput=False)
    bqk_ext = nc.declare_dram_parameter("b_qk", [2 * C, 1], _F32, isOutput=False)
    beff_ext = nc.declare_dram_parameter("b_eff", [C, 1], _F32, isOutput=False)
    ones_ext = nc.declare_dram_parameter("ones", [MC, C, 1], _F32, isOutput=False)
    out_ext = nc.declare_dram_parameter("out", [C, QH], _F32, isOutput=True)

    with (
        nc.allow_low_precision(reason="float32r is 32-bit storage"),
        tile.TileContext(nc) as tc,
        tc.tile_pool(name="consts", bufs=1) as consts,
        # PSUM budget (8 banks): s 2x[128,1024]=4, av 1x[65,1024]=2,
        # pj 2x[<=128,512]=2. qkv psums and the epilogue broadcast share pj.
        tc.tile_pool(name="spool", bufs=2, space="PSUM") as spool,
        tc.tile_pool(name="avpool", bufs=1, space="PSUM") as avpool,
        tc.tile_pool(name="pjpool", bufs=2, space="PSUM") as pjpool,
        tc.tile_pool(name="ebuf", bufs=3) as ebuf,
        tc.tile_pool(name="obuf", bufs=2) as obuf,
    ):
        X = consts.tile([C, N], _F32R, tag="x")
        WQK = consts.tile([C, 2 * C], _F32R, tag="wqk")
        WVP = consts.tile([C, C], _F32R, tag="wvp")
        BQK = consts.tile([2 * C, 1], _F32, tag="bqk")
        BK = consts.tile([C, 1], _F32, tag="bk")
        BEFF = consts.tile([C, 1], _F32, tag="beff")
        ONES1 = consts.tile([1, C], _F32R, tag="ones1")
        Q = consts.tile([C, QH], _F32R, tag="q")
        K = consts.tile([C, N], _F32R, tag="k")
        VT = consts.tile([MC, NMC * 65], _F32R, tag="vt")
        VT3 = VT.rearrange("p (n c) -> p n c", c=65)

        # weights/biases lead each DMA queue, then x chunks interleave
        # across the gpsimd and sync queues
        def dma_x(eng, j):
            eng.dma_start(
                out=X[:, j * 512 : (j + 1) * 512],
                in_=_r(x_ext[:, j * 512 : (j + 1) * 512]),
            )

        nc.gpsimd.dma_start(out=WVP, in_=_r(wvpT_ext[:, :]))
        dma_x(nc.gpsimd, 0)
        nc.sync.dma_start(out=WQK, in_=_r(wqkT_ext[:, :]))
        nc.sync.dma_start(out=BQK, in_=bqk_ext[:, :])
        nc.sync.dma_start(out=BK, in_=bqk_ext[C : 2 * C, :])
        dma_x(nc.sync, 1)
        dma_x(nc.gpsimd, 2)
        nc.sync.dma_start(out=VT3[:, :, 64:65], in_=_r(ones_ext[:, 0:NMC, :]))
        dma_x(nc.sync, 3)
        dma_x(nc.gpsimd, 4)
        nc.sync.dma_start(out=ONES1, in_=_r(ones_ext[0:1, :, 0]))
        nc.sync.dma_start(out=BEFF, in_=beff_ext[:, :])
        dma_x(nc.sync, 5)
        dma_x(nc.gpsimd, 6)
        dma_x(nc.sync, 7)
        # preload the Exp table (1283ns) while DMAs are in flight
        WARM = consts.tile([1, 1], _F32, tag="warm")
        nc.vector.memset(WARM, 0.0)
        nc.scalar.activation(WARM, WARM, _EXP, bias=0.0, scale=1.0)

        # ---- projections, emitted just-in-time inside the attention loop
        # so no engine queue stalls on a not-yet-DMAed x chunk ----
        def emit_qp(j):
            lo, hi = j * 512, (j + 1) * 512
            ps = pjpool.tile([C, 512], _F32, tag="pj")
            nc.tensor.matmul(ps, WQK[:, 0:C], X[:, lo:hi], start=True, stop=True)
            nc.vector.tensor_scalar_add(Q[:, lo:hi], ps, BQK[0:C, :])

        def emit_kp(j):
            lo, hi = j * 512, (j + 1) * 512
            ps = pjpool.tile([C, 512], _F32, tag="pj")
            nc.tensor.matmul(
                ps, WQK[:, C : 2 * C], X[:, lo:hi], start=True, stop=True
            )
            nc.vector.tensor_scalar_add(K[:, lo:hi], ps, BK)

        def emit_vp(g):
            # projected v, transposed, 4 chunks per psum tile + 1 strided copy
            ps = pjpool.tile([MC, 4, C], _F32, tag="pj")
            for i in range(4):
                m = g * 4 + i
                nc.tensor.matmul(
                    ps[:, i, :],
                    X[:, m * MC : (m + 1) * MC],
                    WVP,
                    start=True,
                    stop=True,
                )
            nc.vector.tensor_copy(VT3[:, g * 4 : (g + 1) * 4, 0:C], ps)

        emit_qp(0)
        emit_qp(1)
        emit_kp(0)
        emit_vp(0)
        # one iteration of headroom: projections for chunk group j+1 are
        # emitted after main-loop iteration 4*j
        hooks = {}
        for j in range(1, 8):
            hooks.setdefault(4 * (j - 1), []).append((emit_kp, j))
        for g in range(1, 8):
            hooks.setdefault(4 * (g - 1) + 1, []).append((emit_vp, g))
        hooks.setdefault(2, []).append((emit_qp, 2))
        hooks.setdefault(3, []).append((emit_qp, 3))

        # ---- attention ----
        def epilogue(qb, pav, esz, ucopy_engine):
            q0 = qb * QB
            for h in range(0, QB, esz):
                U = obuf.tile([C + 1, esz], _F32, tag="u")
                ucopy_engine(U, pav[:, h : h + esz])
                R1 = obuf.tile([1, esz], _F32R, tag="r1")
                nc.vector.reciprocal(R1, U[C : C + 1, :])
                pb = pjpool.tile([C, esz], _F32, tag="pj")
                nc.tensor.matmul(pb, ONES1, R1, start=True, stop=True)
                UN = obuf.tile([C, esz], _F32, tag="un")
                nc.vector.tensor_mul(UN, U[0:C, :], pb)
                O = obuf.tile([C, esz], _F32, tag="o")
                nc.vector.scalar_tensor_tensor(
                    out=O,
                    in0=UN,
                    scalar=BEFF,
                    in1=X[:, q0 + h : q0 + h + esz],
                    op0=_ADD,
                    op1=_ADD,
                )
                nc.sync.dma_start(out=out_ext[:, q0 + h : q0 + h + esz], in_=O)

        pending = None  # (qb, pav) whose epilogue is emitted into the next block
        for qb in range(NQB):
            q0 = qb * QB
            pav = avpool.tile([C + 1, QB], _F32, tag="av")
            for m in range(NMC):
                pss = spool.tile([MC, QB], _F32, tag="s")
                for h in (0, 512):
                    nc.tensor.matmul(
                        pss[:, h : h + 512],
                        K[:, m * MC : (m + 1) * MC],
                        Q[:, q0 + h : q0 + h + 512],
                        start=True,
                        stop=True,
                    )
                E = ebuf.tile([MC, QB], _F32R, tag="e")
                nc.scalar.activation(E, pss, _EXP, bias=0.0, scale=0.125)
                for h in (0, 512):
                    nc.tensor.matmul(
                        pav[:, h : h + 512],
                        VT[:, m * 65 : (m + 1) * 65],
                        E[:, h : h + 512],
                        start=(m == 0),
                        stop=(m == NMC - 1),
                    )
                if qb == 0:
                    for fn, arg in hooks.get(m, ()):
                        fn(arg)
                if m == 4 and pending is not None:
                    # previous block's epilogue rides behind this block's
                    # first few iterations in every engine queue
                    epilogue(*pending, esz=512, ucopy_engine=nc.vector.tensor_copy)
                    pending = None
            pending = (qb, pav)
        # final epilogue: fine-grained and with the psum->sbuf copy on the
        # (now idle) scalar engine so the DVE chain pipelines
        epilogue(*pending, esz=256, ucopy_engine=nc.scalar.copy)

    _split_excess_waits(nc)
    return nc


_GRAPH_CACHE = {}


def _get_graph():
    if "nc" not in _GRAPH_CACHE:
        _GRAPH_CACHE["nc"] = build_graph()
    return _GRAPH_CACHE["nc"]


_ONES = np.ones((MC, C, 1), dtype=np.float32)


def make_in_maps(x, w_qkv, b_qkv, w_proj, b_proj):
    xf = np.ascontiguousarray(np.asarray(x, dtype=np.float32).reshape(B, C, N))
    w_qkv = np.asarray(w_qkv, dtype=np.float32)
    b_qkv = np.asarray(b_qkv, dtype=np.float32)
    w_proj = np.asarray(w_proj, dtype=np.float32)
    b_proj = np.asarray(b_proj, dtype=np.float32)

    w_qkT = np.ascontiguousarray(w_qkv[0 : 2 * C].T)
    # fold the output projection into the v projection (weight prep):
    # w_proj @ (w_v @ x) == (w_proj @ w_v) @ x
    w_vpT = np.ascontiguousarray((w_proj @ w_qkv[2 * C :]).T.astype(np.float32))
    b_qk = np.ascontiguousarray(b_qkv[0 : 2 * C].reshape(2 * C, 1))
    # v/proj biases fold to one vector because softmax rows sum to 1
    b_eff = (w_proj @ b_qkv[2 * C :] + b_proj).reshape(C, 1).astype(np.float32)

    in_maps = []
    for core in range(8):
        b, h = divmod(core, 2)
        # rotate tokens so this core's queries are columns 0:QH
        xr = np.ascontiguousarray(np.roll(xf[b], -h * QH, axis=1))
        in_maps.append(
            {
                "x": xr,
                "w_qkT": w_qkT,
                "w_vpT": w_vpT,
                "b_qk": b_qk,
                "b_eff": b_eff,
                "ones": _ONES,
            }
        )
    return in_maps


def kernel(x, w_qkv, b_qkv, w_proj, b_proj):
    x = np.asarray(x)
    nc = _get_graph()
    in_maps = make_in_maps(x, w_qkv, b_qkv, w_proj, b_proj)
    res = run_bass_kernel_spmd(nc, in_maps, core_ids=list(range(8)))
    out = np.empty((B, C, N), dtype=np.float32)
    for core in range(8):
        b, h = divmod(core, 2)
        out[b][:, h * QH : (h + 1) * QH] = res.results[core]["out"]
    return out.reshape(x.shape).astype(np.float32)
